# revision 2
# baseline (speedup 1.0000x reference)
"""Trainium2 Bass kernel for the GODEFunc graph-ODE message-passing module.

Math (per batch b):
    xa   = sum_k conv_w[k] * (adj[k] @ x[b]) + conv_b
    W'   = (w * clip(d,0,1)) @ w.T
    out  = tanh(0.5*sigmoid(alpha) * xa - 2*x[b] + x[b] @ W' + x0[b]*sigmoid(beta))

Sharding: output rows (nodes) split across 8 cores; each core computes its
1024-row slice for all batches.  No collectives.

Key layout decision: the host stages adj TRANSPOSED and 128-row-tiled
(adjt[k, p, mc, n] = adj[k, rows0+n, mc*128+p]) so the contraction dim m
lands on SBUF partitions naturally.  This removes all on-chip PE
transposes (the old bottleneck: ~405us of TensorE busy) and gives every
DMA >=4KB contiguous per-partition lines.

Per-core kernel structure:
  - adjt streams in as bf16 (cast during SWDGE DMA), one [128, 4096]
    tile per (k, mg) group.
  - DVE combines k=0,1 with conv_w scalars -> a single lhsT stream.
  - 512 matmuls [128,128].T @ x_res[128, (b f)] accumulate into 8 PSUM
    row-tile regions over the full contraction depth.
  - xw = x @ (W'-2I) via host-staged x^T tiles (no transposes), W' built
    from host-staged w^T (no transposes).
  - Epilogue: out = tanh(0.5*sig(alpha)*(psum + conv_b) + xw + x0*sig(beta)).
"""

import sys

for _p in ("/opt/trn_rl_repo",):
    if _p not in sys.path:
        sys.path.insert(0, _p)

from contextlib import ExitStack

import numpy as np

import concourse.bass as bass
import concourse.mybir as mybir
import concourse.tile as tile
from concourse import bacc
from concourse.bass_utils import run_bass_kernel_spmd
from concourse.masks import make_identity

dt = mybir.dt
AF = mybir.ActivationFunctionType
ALU = mybir.AluOpType

B, N, F, K = 4, 8192, 64, 2
N_CORES = 8
P = 128

# adj tiles arrive in HBM as fp32 and are cast to bf16 during the SWDGE DMA.
ADJ_BF16_HOST = False  # if True, host stages adjt already in bf16 (HWDGE DMA)


def build_kernel(n=N, n_cores=N_CORES, b=B, f=F, k_dim=K, mg_mc=4,
                 adj_bf16_host=ADJ_BF16_HOST):
    ns = n // n_cores          # rows per core (1024)
    nt_cnt = ns // P           # output row tiles per core (8)
    mc_cnt = n // P            # contraction chunks (64)
    nmg = mc_cnt // mg_mc      # adj DMA groups along contraction dim (16)
    bf = b * f

    nc = bacc.Bacc(None, target_bir_lowering=False, debug=False)

    adj_dt = dt.bfloat16 if adj_bf16_host else dt.float32
    # adjt[k, p, mc, n] = adj[k, rows0+n, mc*128+p]
    adjt = nc.dram_tensor("adjt", [k_dim, P, mc_cnt, ns], adj_dt,
                          kind="ExternalInput")
    # xres_in[p, mc, b, f] = x[b, mc*128+p, f]
    xres_in = nc.dram_tensor("xres_in", [P, mc_cnt, b, f], dt.float32,
                             kind="ExternalInput")
    # xt_rows[f, b, n] = x[b, rows0+n, f]
    xt_rows = nc.dram_tensor("xt_rows", [f, b, ns], dt.float32,
                             kind="ExternalInput")
    # x0t_in[p, nt, b, f] = x0[b, rows0+nt*128+p, f]
    x0t_in = nc.dram_tensor("x0t_in", [P, nt_cnt, b, f], dt.float32,
                            kind="ExternalInput")
    # ab_t[p, 0:nt] = alpha rows tile; ab_t[p, nt:2nt] = beta rows tile
    ab_t = nc.dram_tensor("ab_t", [P, 2 * nt_cnt], dt.float32,
                          kind="ExternalInput")
    wT = nc.dram_tensor("wT", [f, f], dt.float32, kind="ExternalInput")
    d = nc.dram_tensor("d", [f], dt.float32, kind="ExternalInput")
    conv_w = nc.dram_tensor("conv_w", [k_dim], dt.float32, kind="ExternalInput")
    conv_b = nc.dram_tensor("conv_b", [1], dt.float32, kind="ExternalInput")
    # y_t[p, nt, b, f] = out[b, rows0+nt*128+p, f]
    y_t = nc.dram_tensor("y_t", [P, nt_cnt, b, f], dt.float32,
                         kind="ExternalOutput")

    with tile.TileContext(nc) as tc, ExitStack() as ctx:
        const = ctx.enter_context(tc.tile_pool(name="const", bufs=1))
        xres = ctx.enter_context(tc.tile_pool(name="xres", bufs=1))
        adj_pool = ctx.enter_context(tc.tile_pool(name="adjp", bufs=2))
        comb_pool = ctx.enter_context(tc.tile_pool(name="combp", bufs=2))
        work = ctx.enter_context(tc.tile_pool(name="work", bufs=2))
        outp = ctx.enter_context(tc.tile_pool(name="outp", bufs=3))
        keep = ctx.enter_context(tc.tile_pool(name="keep", bufs=1))
        psy = ctx.enter_context(tc.tile_pool(name="psy", bufs=1, space="PSUM"))
        paux = ctx.enter_context(tc.tile_pool(name="paux", bufs=2, space="PSUM"))

        # ---------------- main-stream DMAs issued first ----------------
        # resident x, all batches, contraction layout, bf16 (one big DMA)
        x4 = xres.tile([P, mc_cnt, b, f], dt.bfloat16, tag="x4")
        nc.gpsimd.dma_start(out=x4[:], in_=xres_in[:, :, :, :])

        a_tiles = {}  # (mg, kk) -> tile, prefetched by pool depth
        mgw = mg_mc * ns

        # ---------------- constants / gates ----------------
        ident_f = const.tile([f, f], dt.float32, tag="ident_f")
        make_identity(nc, ident_f[:])

        wT_sb = const.tile([f, f], dt.float32, tag="wT_sb")
        nc.sync.dma_start(out=wT_sb[:], in_=wT[:, :])
        d_sb = const.tile([f, 1], dt.float32, tag="d_sb")
        nc.sync.dma_start(out=d_sb[:], in_=d[:, None])
        cw_sb = const.tile([P, k_dim], dt.float32, tag="cw_sb")
        nc.sync.dma_start(out=cw_sb[:], in_=conv_w[None, :].to_broadcast((P, k_dim)))
        cb_sb = const.tile([P, 1], dt.float32, tag="cb_sb")
        nc.sync.dma_start(out=cb_sb[:], in_=conv_b[None, :].to_broadcast((P, 1)))
        ab_sb = const.tile([P, 2 * nt_cnt], dt.float32, tag="ab_sb")
        nc.sync.dma_start(out=ab_sb[:], in_=ab_t[:, :])

        sigab = const.tile([P, 2 * nt_cnt], dt.float32, tag="sigab")
        nc.scalar.activation(sigab[:], ab_sb[:], AF.Sigmoid)
        siga = sigab[:, 0:nt_cnt]
        sigb = sigab[:, nt_cnt : 2 * nt_cnt]
        # vec_a[p, t] = 0.5 * sigmoid(alpha)
        vec_a = const.tile([P, nt_cnt], dt.float32, tag="vec_a")
        nc.vector.tensor_scalar(vec_a[:], siga, 0.5, None, ALU.mult)
        # bias_cb[p, t] = 0.5 * sigmoid(alpha) * conv_b
        bias_cb = const.tile([P, nt_cnt], dt.float32, tag="bias_cb")
        nc.vector.tensor_scalar(bias_cb[:], vec_a[:], cb_sb[:, 0:1], None, ALU.mult)

        # ---------------- W' = (w * clip(d,0,1)) @ w.T - 2I ----------------
        dc = const.tile([f, 1], dt.float32, tag="dc")
        nc.vector.tensor_scalar(dc[:], d_sb[:], 0.0, 1.0, ALU.max, ALU.min)
        wdcT = const.tile([f, f], dt.float32, tag="wdcT")
        nc.vector.tensor_scalar(wdcT[:], wT_sb[:], dc[:], None, ALU.mult)
        pw = paux.tile([f, f], dt.float32, tag="pw")
        nc.tensor.matmul(pw[:], wdcT[:], wT_sb[:], start=True, stop=True)
        wp = const.tile([f, f], dt.float32, tag="wp")
        nc.vector.scalar_tensor_tensor(
            wp[:], ident_f[:], -2.0, pw[:], ALU.mult, ALU.add
        )

        # ---------------- xt rows (for xw), x0 ----------------
        xt_sb = const.tile([f, b, ns], dt.float32, tag="xt_sb")
        nc.sync.dma_start(out=xt_sb[:], in_=xt_rows[:, :, :])
        x0_sb = const.tile([P, nt_cnt, b, f], dt.float32, tag="x0_sb")
        nc.sync.dma_start(out=x0_sb[:], in_=x0t_in[:, :, :, :])

        # ---------------- psum accumulators: two row-tiles per bank ----------
        n_banks = (nt_cnt + 1) // 2
        psum_y = [
            psy.tile([P, 2 * bf], dt.float32, tag=f"y{i}", name=f"psum_y{i}")
            for i in range(n_banks)
        ]

        def y_region(ntt):
            return psum_y[ntt // 2][:, (ntt % 2) * bf : (ntt % 2 + 1) * bf]

        # ---------------- xw = x_rows @ (W'-2I) + x0*sigmoid(beta) ----------
        xwx0 = []
        for ntt in range(nt_cnt):
            pxw = paux.tile([P, bf], dt.float32, tag="pxw")
            for bb in range(b):
                nc.tensor.matmul(
                    pxw[:, bb * f : (bb + 1) * f],
                    xt_sb[:, bb, ntt * P : (ntt + 1) * P],
                    wp[:],
                    start=True,
                    stop=True,
                )
            acc = keep.tile([P, bf], dt.float32, tag=f"xwx0_{ntt}")
            nc.vector.scalar_tensor_tensor(
                acc[:],
                x0_sb[:, ntt, :, :].rearrange("p b f -> p (b f)"),
                sigb[:, ntt : ntt + 1],
                pxw[:],
                ALU.mult,
                ALU.add,
            )
            xwx0.append(acc)

        # ---------------- main loop: stream adjt, k-combine, matmul ----------
        for mg in range(nmg):
            cs = slice(mg * mg_mc, (mg + 1) * mg_mc)
            a_k = []
            for kk in range(k_dim):
                a_t = adj_pool.tile([P, mg_mc * ns], dt.bfloat16,
                                    tag=f"adj{kk}", name=f"adj_t{kk}")
                if adj_bf16_host:
                    nc.sync.dma_start(
                        out=a_t[:].rearrange("p (c n) -> p c n", c=mg_mc),
                        in_=adjt[kk, :, cs, :],
                    )
                else:
                    nc.gpsimd.dma_start(
                        out=a_t[:].rearrange("p (c n) -> p c n", c=mg_mc),
                        in_=adjt[kk, :, cs, :],
                    )
                a_k.append(a_t)
            # comb = conv_w[0]*a0 + conv_w[1]*a1  (bf16, DVE)
            nc.vector.tensor_scalar(
                a_k[1][:], a_k[1][:], cw_sb[:, 1:2], None, ALU.mult
            )
            comb = comb_pool.tile([P, mg_mc * ns], dt.bfloat16, tag="comb")
            nc.vector.scalar_tensor_tensor(
                comb[:], a_k[0][:], cw_sb[:, 0:1], a_k[1][:], ALU.mult, ALU.add
            )
            for c in range(mg_mc):
                mc = mg * mg_mc + c
                for ntt in range(nt_cnt):
                    nc.tensor.matmul(
                        y_region(ntt),
                        comb[:, c * ns + ntt * P : c * ns + (ntt + 1) * P],
                        x4[:, mc, :, :],
                        start=(mc == 0),
                        stop=(mc == mc_cnt - 1),
                        skip_group_check=True,
                    )

        # ---------------- epilogue: tanh(0.5*siga*(y+cb) + xw + x0*sigb) -----
        for ntt in range(nt_cnt):
            acc = outp.tile([P, bf], dt.float32, tag="eacc")
            nc.vector.scalar_tensor_tensor(
                acc[:], y_region(ntt), vec_a[:, ntt : ntt + 1], xwx0[ntt][:],
                ALU.mult, ALU.add,
            )
            outt = outp.tile([P, bf], dt.float32, tag="outt")
            nc.scalar.activation(
                outt[:], acc[:], AF.Tanh, bias=bias_cb[:, ntt : ntt + 1]
            )
            nc.sync.dma_start(
                out=y_t[:, ntt, :, :],
                in_=outt[:].rearrange("p (b f) -> p b f", b=b),
            )

    nc.finalize()
    return nc


_NC_CACHE = {}


def _get_nc(key=(N, N_CORES, B, F, K)):
    if key not in _NC_CACHE:
        _NC_CACHE[key] = build_kernel(*key[:5]) if len(key) > 5 else build_kernel(
            n=key[0], n_cores=key[1], b=key[2], f=key[3], k_dim=key[4]
        )
    return _NC_CACHE[key]


def make_in_maps(x, x0, adj, alpha, beta, w, d, conv_w, conv_b, n_cores=N_CORES):
    """Host-side staging: slice rows per core and retile/transpose (pure
    layout transforms; values unchanged)."""
    k_dim, n, _ = adj.shape
    b, _, f = x.shape
    ns = n // n_cores
    nt_cnt = ns // P
    mc_cnt = n // P
    f32 = np.float32
    adj = np.asarray(adj, dtype=f32)
    x = np.asarray(x, dtype=f32)
    x0 = np.asarray(x0, dtype=f32)

    # xres_in[p, mc, b, f] = x[b, mc*128+p, f]   (shared by all cores)
    xres_in = np.ascontiguousarray(
        x.reshape(b, mc_cnt, P, f).transpose(2, 1, 0, 3)
    )

    wT = np.ascontiguousarray(np.asarray(w, dtype=f32).T)

    in_maps = []
    for c in range(n_cores):
        rows = slice(c * ns, (c + 1) * ns)
        ac = adj[:, rows, :]  # [k, ns, n] view
        # adjt[k, p, mc, nn] = ac[k, nn, mc*128+p]
        s0, s1, s2 = ac.strides
        adjt = np.lib.stride_tricks.as_strided(
            ac, shape=(k_dim, P, mc_cnt, ns), strides=(s0, s2, P * s2, s1)
        )
        adjt = np.ascontiguousarray(adjt)
        if ADJ_BF16_HOST:
            import ml_dtypes

            adjt = adjt.astype(ml_dtypes.bfloat16)

        xr = x[:, rows, :]  # [b, ns, f]
        xt_rows = np.ascontiguousarray(xr.transpose(2, 0, 1))  # [f, b, ns]
        x0t_in = np.ascontiguousarray(
            x0[:, rows, :].reshape(b, nt_cnt, P, f).transpose(2, 1, 0, 3)
        )
        ab_t = np.empty((P, 2 * nt_cnt), dtype=f32)
        ab_t[:, :nt_cnt] = alpha[rows].reshape(nt_cnt, P).T
        ab_t[:, nt_cnt:] = beta[rows].reshape(nt_cnt, P).T

        in_maps.append(
            {
                "adjt": adjt,
                "xres_in": xres_in,
                "xt_rows": xt_rows,
                "x0t_in": x0t_in,
                "ab_t": np.ascontiguousarray(ab_t),
                "wT": wT,
                "d": np.ascontiguousarray(d, dtype=f32),
                "conv_w": np.ascontiguousarray(conv_w, dtype=f32),
                "conv_b": np.ascontiguousarray(conv_b, dtype=f32),
            }
        )
    return in_maps


def assemble_output(per_core_y, n_cores=N_CORES):
    """y_t[p, nt, b, f] per core -> full [b, n, f]."""
    parts = []
    for c in range(n_cores):
        yt = per_core_y[c]  # [P, nt, b, f]
        p_, nt_, b_, f_ = yt.shape
        parts.append(yt.transpose(2, 1, 0, 3).reshape(b_, nt_ * p_, f_))
    return np.concatenate(parts, axis=1)


def kernel(x, x0, adj, alpha, beta, w, d, conv_w, conv_b):
    x = np.asarray(x)
    x0 = np.asarray(x0)
    adj = np.asarray(adj)
    alpha = np.asarray(alpha)
    beta = np.asarray(beta)
    w = np.asarray(w)
    d = np.asarray(d)
    conv_w = np.asarray(conv_w)
    conv_b = np.asarray(conv_b)

    nc = _get_nc()
    in_maps = make_in_maps(x, x0, adj, alpha, beta, w, d, conv_w, conv_b)
    res = run_bass_kernel_spmd(nc, in_maps, core_ids=list(range(N_CORES)))
    out = assemble_output([res.results[c]["y_t"] for c in range(N_CORES)])
    return out.astype(np.float32)


# revision 3
# speedup vs baseline: 1.0158x; 1.0158x over previous
"""Trainium2 Bass kernel for the GODEFunc graph-ODE message-passing module.

Math (per batch b):
    xa   = sum_k conv_w[k] * (adj[k] @ x[b]) + conv_b
    W'   = (w * clip(d,0,1)) @ w.T          (symmetric)
    out  = tanh(0.5*sigmoid(alpha) * xa - 2*x[b] + x[b] @ W' + x0[b]*sigmoid(beta))

Sharding: output rows (nodes) split across 8 cores; each core computes its
1024-row slice for all batches.  No collectives.

Layout decisions:
  - Host stages adj TRANSPOSED and 128-row tiled
    (adjt[k, p, mc, n] = adj[k, rows0+n, mc*128+p]) so the contraction dim m
    lands on SBUF partitions naturally -> zero on-chip transposes.
  - The matmul runs x-STATIONARY: lhsT = x4[mc] column pairs (128 rows of
    contraction x 128 (b,f) columns), moving operand = the combined adj
    stream at N=512.  256 big matmuls instead of 512 small ones; the
    output is produced transposed ([bf, n]) and the host un-transposes.
  - Each PSUM accumulation group owns a full 2KB bank (has_written/"zero
    region" flags are per-bank on TRN2, so two groups must never share a
    bank across a start boundary).
  - k=0,1 combined on DVE with conv_w scalars; 0.5*sigmoid(alpha),
    sigmoid(beta) applied in the epilogue via partition-broadcast tiles.
"""

import sys

for _p in ("/opt/trn_rl_repo",):
    if _p not in sys.path:
        sys.path.insert(0, _p)

from contextlib import ExitStack

import numpy as np

import concourse.bass as bass
import concourse.mybir as mybir
import concourse.tile as tile
from concourse import bacc
from concourse.bass_utils import run_bass_kernel_spmd
from concourse.masks import make_identity

dt = mybir.dt
AF = mybir.ActivationFunctionType
ALU = mybir.AluOpType

B, N, F, K = 4, 8192, 64, 2
N_CORES = 8
P = 128

ADJ_BF16_HOST = False  # if True, host stages adjt already in bf16 (HWDGE DMA)


def build_kernel(n=N, n_cores=N_CORES, b=B, f=F, k_dim=K, mg_mc=4,
                 adj_bf16_host=ADJ_BF16_HOST):
    ns = n // n_cores          # rows per core (1024)
    mc_cnt = n // P            # contraction chunks (64)
    nmg = mc_cnt // mg_mc      # adj DMA groups along contraction dim
    bf = b * f
    nh = bf // P               # output partition halves (2)
    nw = ns // 512             # 512-col slices of the row range (2)

    nc = bacc.Bacc(None, target_bir_lowering=False, debug=False)

    adj_dt = dt.bfloat16 if adj_bf16_host else dt.float32
    # adjt[k, p, mc, n] = adj[k, rows0+n, mc*128+p]
    adjt = nc.dram_tensor("adjt", [k_dim, P, mc_cnt, ns], adj_dt,
                          kind="ExternalInput")
    # xres_in[p, mc, b, f] = x[b, mc*128+p, f]
    xres_in = nc.dram_tensor("xres_in", [P, mc_cnt, b, f], dt.float32,
                             kind="ExternalInput")
    # xt_rows[f, b, n] = x[b, rows0+n, f]
    xt_rows = nc.dram_tensor("xt_rows", [f, b, ns], dt.float32,
                             kind="ExternalInput")
    # x0t_in[p, h, n] = x0[2h + p//64, rows0+n, p%64]
    x0t_in = nc.dram_tensor("x0t_in", [P, nh, ns], dt.float32,
                            kind="ExternalInput")
    # ab_rows[0, n] = alpha[rows0+n]; ab_rows[1, n] = beta[rows0+n]
    ab_rows = nc.dram_tensor("ab_rows", [2, ns], dt.float32,
                             kind="ExternalInput")
    wT = nc.dram_tensor("wT", [f, f], dt.float32, kind="ExternalInput")
    d = nc.dram_tensor("d", [f], dt.float32, kind="ExternalInput")
    conv_w = nc.dram_tensor("conv_w", [k_dim], dt.float32, kind="ExternalInput")
    conv_b = nc.dram_tensor("conv_b", [1], dt.float32, kind="ExternalInput")
    # y_T[h, p, n] = out[2h + p//64, rows0+n, p%64]
    y_T = nc.dram_tensor("y_T", [nh, P, ns], dt.float32,
                         kind="ExternalOutput")

    with tile.TileContext(nc) as tc, ExitStack() as ctx:
        const = ctx.enter_context(tc.tile_pool(name="const", bufs=1))
        xres = ctx.enter_context(tc.tile_pool(name="xres", bufs=1))
        adj_pool = ctx.enter_context(tc.tile_pool(name="adjp", bufs=2))
        comb_pool = ctx.enter_context(tc.tile_pool(name="combp", bufs=2))
        work = ctx.enter_context(tc.tile_pool(name="work", bufs=2))
        outp = ctx.enter_context(tc.tile_pool(name="outp", bufs=2))
        psy = ctx.enter_context(tc.tile_pool(name="psy", bufs=1, space="PSUM"))
        ppxw = ctx.enter_context(tc.tile_pool(name="ppxw", bufs=1, space="PSUM"))

        # ---------------- main-stream DMAs issued first ----------------
        # resident x, all batches, contraction layout, bf16 (one big DMA)
        x4 = xres.tile([P, mc_cnt, b, f], dt.bfloat16, tag="x4")
        nc.gpsimd.dma_start(out=x4[:], in_=xres_in[:, :, :, :])

        # ---------------- constants / gates ----------------
        ident_f = const.tile([f, f], dt.float32, tag="ident_f")
        make_identity(nc, ident_f[:])

        wT_sb = const.tile([f, f], dt.float32, tag="wT_sb")
        nc.sync.dma_start(out=wT_sb[:], in_=wT[:, :])
        d_sb = const.tile([f, 1], dt.float32, tag="d_sb")
        nc.sync.dma_start(out=d_sb[:], in_=d[:, None])
        cw_sb = const.tile([P, k_dim], dt.float32, tag="cw_sb")
        nc.sync.dma_start(out=cw_sb[:], in_=conv_w[None, :].to_broadcast((P, k_dim)))
        cb_sb = const.tile([P, 1], dt.float32, tag="cb_sb")
        nc.sync.dma_start(out=cb_sb[:], in_=conv_b[None, :].to_broadcast((P, 1)))

        # alpha/beta broadcast down partitions: [P, ns]
        aB = const.tile([P, ns], dt.float32, tag="aB")
        nc.sync.dma_start(out=aB[:], in_=ab_rows[0:1, :].to_broadcast((P, ns)))
        bB = const.tile([P, ns], dt.float32, tag="bB")
        nc.sync.dma_start(out=bB[:], in_=ab_rows[1:2, :].to_broadcast((P, ns)))
        siga05B = const.tile([P, ns], dt.float32, tag="siga05B")
        nc.scalar.activation(siga05B[:], aB[:], AF.Sigmoid)
        nc.vector.tensor_scalar(siga05B[:], siga05B[:], 0.5, None, ALU.mult)
        sigbB = const.tile([P, ns], dt.float32, tag="sigbB")
        nc.scalar.activation(sigbB[:], bB[:], AF.Sigmoid)

        # ---------------- xt rows (for xw), x0 ----------------
        xt_sb = const.tile([f, b, ns], dt.float32, tag="xt_sb")
        nc.sync.dma_start(out=xt_sb[:], in_=xt_rows[:, :, :])
        x0_sb = const.tile([P, nh, ns], dt.float32, tag="x0_sb")
        nc.sync.dma_start(out=x0_sb[:], in_=x0t_in[:, :, :])

        # ---------------- PSUM: 4 banks y, 4 banks pxw ----------------
        psum_T = [psy.tile([P, ns], dt.float32, tag=f"yT{h}", name=f"psum_T{h}")
                  for h in range(nh)]
        pxw_T = [ppxw.tile([P, ns], dt.float32, tag=f"pxw{h}", name=f"pxw_T{h}")
                 for h in range(nh)]

        # ---------------- W' = (w * clip(d,0,1)) @ w.T - 2I ----------------
        dc = const.tile([f, 1], dt.float32, tag="dc")
        nc.vector.tensor_scalar(dc[:], d_sb[:], 0.0, 1.0, ALU.max, ALU.min)
        wdcT = const.tile([f, f], dt.float32, tag="wdcT")
        nc.vector.tensor_scalar(wdcT[:], wT_sb[:], dc[:], None, ALU.mult)
        pw = pxw_T[0][0:f, 0:f]  # borrow a pxw bank region; consumed before pxw
        nc.tensor.matmul(pw, wdcT[:], wT_sb[:], start=True, stop=True)
        wp = const.tile([f, f], dt.float32, tag="wp")
        nc.vector.scalar_tensor_tensor(
            wp[:], ident_f[:], -2.0, pw, ALU.mult, ALU.add
        )

        # ---------------- pxw_T[h] = (x_rows @ (W'-2I)).T, transposed layout --
        # wp is symmetric, so it serves directly as lhsT.
        for h in range(nh):
            for bb in (2 * h, 2 * h + 1):
                base = (bb % 2) * f
                for s in range(nw):
                    nc.tensor.matmul(
                        pxw_T[h][base : base + f, s * 512 : (s + 1) * 512],
                        wp[:],
                        xt_sb[:, bb, s * 512 : (s + 1) * 512],
                        start=True,
                        stop=True,
                    )

        # ---------------- main loop: stream adjt, k-combine, matmul ----------
        for mg in range(nmg):
            cs = slice(mg * mg_mc, (mg + 1) * mg_mc)
            a_k = []
            for kk in range(k_dim):
                a_t = adj_pool.tile([P, mg_mc * ns], dt.bfloat16,
                                    tag=f"adj{kk}", name=f"adj_t{kk}")
                eng = nc.sync if adj_bf16_host else nc.gpsimd
                eng.dma_start(
                    out=a_t[:].rearrange("p (c n) -> p c n", c=mg_mc),
                    in_=adjt[kk, :, cs, :],
                )
                a_k.append(a_t)
            # comb = conv_w[0]*a0 + conv_w[1]*a1  (bf16, DVE)
            nc.vector.tensor_scalar(
                a_k[1][:], a_k[1][:], cw_sb[:, 1:2], None, ALU.mult
            )
            comb = comb_pool.tile([P, mg_mc * ns], dt.bfloat16, tag="comb")
            nc.vector.scalar_tensor_tensor(
                comb[:], a_k[0][:], cw_sb[:, 0:1], a_k[1][:], ALU.mult, ALU.add
            )
            for c in range(mg_mc):
                mc = mg * mg_mc + c
                for h in range(nh):
                    lhsT = x4[:, mc, 2 * h : 2 * h + 2, :].rearrange(
                        "p b f -> p (b f)"
                    )
                    for s in range(nw):
                        nc.tensor.matmul(
                            psum_T[h][:, s * 512 : (s + 1) * 512],
                            lhsT,
                            comb[:, c * ns + s * 512 : c * ns + (s + 1) * 512],
                            start=(mc == 0),
                            stop=(mc == mc_cnt - 1),
                            skip_group_check=True,
                        )

        # ---------------- epilogue ----------------
        # out = tanh(0.5*siga*(y + cb) + xw + x0*sigb), all in [bf, n] layout
        for h in range(nh):
            e1 = work.tile([P, ns], dt.float32, tag="e1")
            nc.vector.tensor_tensor(e1[:], x0_sb[:, h, :], sigbB[:], ALU.mult)
            e2 = work.tile([P, ns], dt.float32, tag="e2")
            nc.vector.tensor_tensor(e2[:], e1[:], pxw_T[h][:], ALU.add)
            e3 = work.tile([P, ns], dt.float32, tag="e3")
            nc.vector.tensor_tensor(e3[:], psum_T[h][:], siga05B[:], ALU.mult)
            e4 = work.tile([P, ns], dt.float32, tag="e4")
            nc.vector.tensor_tensor(e4[:], e3[:], e2[:], ALU.add)
            # + 0.5*siga*conv_b
            e5 = work.tile([P, ns], dt.float32, tag="e5")
            nc.vector.scalar_tensor_tensor(
                e5[:], siga05B[:], cb_sb[:, 0:1], e4[:], ALU.mult, ALU.add
            )
            outt = outp.tile([P, ns], dt.float32, tag="outt")
            nc.scalar.activation(outt[:], e5[:], AF.Tanh)
            nc.sync.dma_start(out=y_T[h, :, :], in_=outt[:])

    nc.finalize()
    return nc


_NC_CACHE = {}


def _get_nc():
    key = (N, N_CORES, B, F, K, ADJ_BF16_HOST)
    if key not in _NC_CACHE:
        _NC_CACHE[key] = build_kernel(
            n=N, n_cores=N_CORES, b=B, f=F, k_dim=K, adj_bf16_host=ADJ_BF16_HOST
        )
    return _NC_CACHE[key]


def make_in_maps(x, x0, adj, alpha, beta, w, d, conv_w, conv_b, n_cores=N_CORES):
    """Host-side staging: slice rows per core and retile/transpose (pure
    layout transforms)."""
    k_dim, n, _ = adj.shape
    b, _, f = x.shape
    ns = n // n_cores
    mc_cnt = n // P
    nh = (b * f) // P
    f32 = np.float32
    adj = np.asarray(adj, dtype=f32)
    x = np.asarray(x, dtype=f32)
    x0 = np.asarray(x0, dtype=f32)
    alpha = np.asarray(alpha, dtype=f32)
    beta = np.asarray(beta, dtype=f32)

    # xres_in[p, mc, b, f] = x[b, mc*128+p, f]   (shared by all cores)
    xres_in = np.ascontiguousarray(
        x.reshape(b, mc_cnt, P, f).transpose(2, 1, 0, 3)
    )
    wT = np.ascontiguousarray(np.asarray(w, dtype=f32).T)

    in_maps = []
    for c in range(n_cores):
        rows = slice(c * ns, (c + 1) * ns)
        ac = adj[:, rows, :]  # [k, ns, n] view
        # adjt[k, p, mc, nn] = ac[k, nn, mc*128+p]
        s0, s1, s2 = ac.strides
        adjt = np.lib.stride_tricks.as_strided(
            ac, shape=(k_dim, P, mc_cnt, ns), strides=(s0, s2, P * s2, s1)
        )
        adjt = np.ascontiguousarray(adjt)
        if ADJ_BF16_HOST:
            import ml_dtypes

            adjt = adjt.astype(ml_dtypes.bfloat16)

        xr = x[:, rows, :]  # [b, ns, f]
        xt_rows = np.ascontiguousarray(xr.transpose(2, 0, 1))  # [f, b, ns]
        # x0t_in[p, h, nn] = x0[2h + p//64, rows0+nn, p%64]
        x0t_in = np.ascontiguousarray(
            x0[:, rows, :].transpose(0, 2, 1).reshape(nh, P, ns).transpose(1, 0, 2)
        )
        ab = np.stack([alpha[rows], beta[rows]], axis=0)

        in_maps.append(
            {
                "adjt": adjt,
                "xres_in": xres_in,
                "xt_rows": xt_rows,
                "x0t_in": x0t_in,
                "ab_rows": np.ascontiguousarray(ab),
                "wT": wT,
                "d": np.ascontiguousarray(d, dtype=f32),
                "conv_w": np.ascontiguousarray(conv_w, dtype=f32),
                "conv_b": np.ascontiguousarray(conv_b, dtype=f32),
            }
        )
    return in_maps


def assemble_output(per_core_y, n_cores=N_CORES):
    """y_T[h, p, n] per core -> full [b, n, f]."""
    parts = []
    for c in range(n_cores):
        yt = per_core_y[c]  # [nh, P, ns]
        nh_, p_, ns_ = yt.shape
        b_ = nh_ * (p_ // 64)
        f_ = 64
        # [nh, P, ns] -> [b, f, ns] -> [b, ns, f]
        parts.append(
            yt.reshape(nh_, p_ // f_, f_, ns_)
            .reshape(b_, f_, ns_)
            .transpose(0, 2, 1)
        )
    return np.concatenate(parts, axis=1)


def kernel(x, x0, adj, alpha, beta, w, d, conv_w, conv_b):
    x = np.asarray(x)
    x0 = np.asarray(x0)
    adj = np.asarray(adj)
    alpha = np.asarray(alpha)
    beta = np.asarray(beta)
    w = np.asarray(w)
    d = np.asarray(d)
    conv_w = np.asarray(conv_w)
    conv_b = np.asarray(conv_b)

    nc = _get_nc()
    in_maps = make_in_maps(x, x0, adj, alpha, beta, w, d, conv_w, conv_b)
    res = run_bass_kernel_spmd(nc, in_maps, core_ids=list(range(N_CORES)))
    out = assemble_output([res.results[c]["y_T"] for c in range(N_CORES)])
    return out.astype(np.float32)


# revision 10
# speedup vs baseline: 1.7457x; 1.7186x over previous
"""Trainium2 Bass kernel for the GODEFunc graph-ODE message-passing module.

Math (per batch b):
    xa   = sum_k conv_w[k] * (adj[k] @ x[b]) + conv_b
    W'   = (w * clip(d,0,1)) @ w.T          (symmetric)
    out  = tanh(0.5*sigmoid(alpha) * xa - 2*x[b] + x[b] @ W' + x0[b]*sigmoid(beta))

Sharding: output rows (nodes) split across 8 cores; each core computes its
1024-row slice for all batches.  No collectives.

Layout decisions:
  - Host stages adj TRANSPOSED and 128-row tiled
    (adjt[k, p, mc, n] = adj[k, rows0+n, mc*128+p]) so the contraction dim m
    lands on SBUF partitions naturally -> zero on-chip transposes.
  - The matmul runs x-STATIONARY: lhsT = x4[mc] column pairs (128 rows of
    contraction x 128 (b,f) columns), moving operand = the combined adj
    stream at N=512.  256 big matmuls instead of 512 small ones; the
    output is produced transposed ([bf, n]) and the host un-transposes.
  - Each PSUM accumulation group owns a full 2KB bank (has_written/"zero
    region" flags are per-bank on TRN2, so two groups must never share a
    bank across a start boundary).
  - k=0,1 combined on DVE with conv_w scalars; 0.5*sigmoid(alpha),
    sigmoid(beta) applied in the epilogue via partition-broadcast tiles.
"""

import sys

for _p in ("/opt/trn_rl_repo",):
    if _p not in sys.path:
        sys.path.insert(0, _p)

from contextlib import ExitStack

import numpy as np

import concourse.bass as bass
import concourse.mybir as mybir
import concourse.tile as tile
from concourse import bacc
from concourse.bass_utils import run_bass_kernel_spmd
from concourse.masks import make_identity

dt = mybir.dt
AF = mybir.ActivationFunctionType
ALU = mybir.AluOpType

B, N, F, K = 4, 8192, 64, 2
N_CORES = 8
P = 128

# The kernel computes the adjacency matmul in bf16 either way (values are
# bit-identical); staging the bf16 cast on the host halves the HBM bytes the
# cores must stream.  The xw path and all gates stay fp32.
ADJ_BF16_HOST = True


def build_kernel(n=N, n_cores=N_CORES, b=B, f=F, k_dim=K, mg_mc=4,
                 adj_bf16_host=ADJ_BF16_HOST):
    ns = n // n_cores          # rows per core (1024)
    mc_cnt = n // P            # contraction chunks (64)
    nmg = mc_cnt // mg_mc      # adj DMA groups along contraction dim
    bf = b * f
    nh = bf // P               # output partition halves (2)
    nw = ns // 512             # 512-col slices of the row range (2)

    nc = bacc.Bacc(None, target_bir_lowering=False, debug=False)

    adj_dt = dt.bfloat16 if adj_bf16_host else dt.float32
    # adjt[k, p, mc, n] = adj[k, rows0+n, mc*128+p]
    adjt = nc.dram_tensor("adjt", [k_dim, P, mc_cnt, ns], adj_dt,
                          kind="ExternalInput")
    # xres_in[p, mc, b, f] = x[b, mc*128+p, f]
    xres_in = nc.dram_tensor("xres_in", [P, mc_cnt, b, f], adj_dt,
                             kind="ExternalInput")
    # xt_rows[f, b, n] = x[b, rows0+n, f]
    xt_rows = nc.dram_tensor("xt_rows", [f, b, ns], dt.float32,
                             kind="ExternalInput")
    # x0t_in[p, h, n] = x0[2h + p//64, rows0+n, p%64]
    x0t_in = nc.dram_tensor("x0t_in", [P, nh, ns], dt.float32,
                            kind="ExternalInput")
    # ab_rows[0, n] = alpha[rows0+n]; ab_rows[1, n] = beta[rows0+n]
    ab_rows = nc.dram_tensor("ab_rows", [2, ns], dt.float32,
                             kind="ExternalInput")
    wT = nc.dram_tensor("wT", [f, f], dt.float32, kind="ExternalInput")
    d = nc.dram_tensor("d", [f], dt.float32, kind="ExternalInput")
    conv_w = nc.dram_tensor("conv_w", [k_dim], dt.float32, kind="ExternalInput")
    conv_b = nc.dram_tensor("conv_b", [1], dt.float32, kind="ExternalInput")
    # y_T[h, p, n] = out[2h + p//64, rows0+n, p%64]
    y_T = nc.dram_tensor("y_T", [nh, P, ns], dt.float32,
                         kind="ExternalOutput")

    with tile.TileContext(nc) as tc, ExitStack() as ctx:
        const = ctx.enter_context(tc.tile_pool(name="const", bufs=1))
        xres = ctx.enter_context(tc.tile_pool(name="xres", bufs=1))
        adj_pool = ctx.enter_context(tc.tile_pool(name="adjp", bufs=3))
        comb_pool = ctx.enter_context(tc.tile_pool(name="combp", bufs=3))
        work = ctx.enter_context(tc.tile_pool(name="work", bufs=3))
        outp = ctx.enter_context(tc.tile_pool(name="outp", bufs=2))
        psy = ctx.enter_context(tc.tile_pool(name="psy", bufs=1, space="PSUM"))
        ppxw = ctx.enter_context(tc.tile_pool(name="ppxw", bufs=1, space="PSUM"))

        # ---------------- main-stream DMAs issued first ----------------
        # resident x, all batches, contraction layout, bf16 (chunked so the
        # first matmuls can start before the whole tensor lands)
        x4 = xres.tile([P, mc_cnt, b, f], dt.bfloat16, tag="x4")
        x4_chunk = mc_cnt // 4
        for xc in range(4):
            xs = slice(xc * x4_chunk, (xc + 1) * x4_chunk)
            if adj_bf16_host:
                nc.sync.dma_start(out=x4[:, xs, :, :], in_=xres_in[:, xs, :, :])
            else:
                nc.gpsimd.dma_start(out=x4[:, xs, :, :], in_=xres_in[:, xs, :, :])

        # ---------------- constants / gates ----------------
        ident_f = const.tile([f, f], dt.float32, tag="ident_f")
        make_identity(nc, ident_f[:])

        wT_sb = const.tile([f, f], dt.float32, tag="wT_sb")
        nc.sync.dma_start(out=wT_sb[:], in_=wT[:, :])
        d_sb = const.tile([f, 1], dt.float32, tag="d_sb")
        nc.sync.dma_start(out=d_sb[:], in_=d[:, None])
        cw_sb = const.tile([P, k_dim], dt.float32, tag="cw_sb")
        nc.sync.dma_start(out=cw_sb[:], in_=conv_w[None, :].to_broadcast((P, k_dim)))
        cb_sb = const.tile([P, 1], dt.float32, tag="cb_sb")
        nc.sync.dma_start(out=cb_sb[:], in_=conv_b[None, :].to_broadcast((P, 1)))

        # alpha/beta broadcast down partitions: [P, ns]
        aB = const.tile([P, ns], dt.float32, tag="aB")
        nc.sync.dma_start(out=aB[:], in_=ab_rows[0:1, :].to_broadcast((P, ns)))
        bB = const.tile([P, ns], dt.float32, tag="bB")
        nc.sync.dma_start(out=bB[:], in_=ab_rows[1:2, :].to_broadcast((P, ns)))
        siga05B = const.tile([P, ns], dt.float32, tag="siga05B")
        nc.scalar.activation(siga05B[:], aB[:], AF.Sigmoid)
        nc.vector.tensor_scalar(siga05B[:], siga05B[:], 0.5, None, ALU.mult)
        sigbB = const.tile([P, ns], dt.float32, tag="sigbB")
        nc.scalar.activation(sigbB[:], bB[:], AF.Sigmoid)

        # ---------------- xt rows (for xw), x0 ----------------
        xt_sb = const.tile([f, b, ns], dt.float32, tag="xt_sb")
        nc.sync.dma_start(out=xt_sb[:], in_=xt_rows[:, :, :])
        x0_sb = const.tile([P, nh, ns], dt.float32, tag="x0_sb")
        nc.sync.dma_start(out=x0_sb[:], in_=x0t_in[:, :, :])

        # ---------------- PSUM: 4 banks y, 4 banks pxw ----------------
        psum_T = [psy.tile([P, ns], dt.float32, tag=f"yT{h}", name=f"psum_T{h}")
                  for h in range(nh)]
        pxw_T = [ppxw.tile([P, ns], dt.float32, tag=f"pxw{h}", name=f"pxw_T{h}")
                 for h in range(nh)]

        # ---------------- W' = (w * clip(d,0,1)) @ w.T - 2I ----------------
        dc = const.tile([f, 1], dt.float32, tag="dc")
        nc.vector.tensor_scalar(dc[:], d_sb[:], 0.0, 1.0, ALU.max, ALU.min)
        wdcT = const.tile([f, f], dt.float32, tag="wdcT")
        nc.vector.tensor_scalar(wdcT[:], wT_sb[:], dc[:], None, ALU.mult)
        pw = pxw_T[0][0:f, 0:f]  # borrow a pxw bank region; consumed before pxw
        nc.tensor.matmul(pw, wdcT[:], wT_sb[:], start=True, stop=True)
        wp = const.tile([f, f], dt.float32, tag="wp")
        nc.vector.scalar_tensor_tensor(
            wp[:], ident_f[:], -2.0, pw, ALU.mult, ALU.add
        )

        # ---------------- pxw_T[h] = (x_rows @ (W'-2I)).T, transposed layout --
        # wp is symmetric, so it serves directly as lhsT.
        for h in range(nh):
            for bb in (2 * h, 2 * h + 1):
                base = (bb % 2) * f
                for s in range(nw):
                    nc.tensor.matmul(
                        pxw_T[h][base : base + f, s * 512 : (s + 1) * 512],
                        wp[:],
                        xt_sb[:, bb, s * 512 : (s + 1) * 512],
                        start=True,
                        stop=True,
                    )

        # ---------------- main loop: stream adjt, k-combine, matmul ----------
        for mg in range(nmg):
            cs = slice(mg * mg_mc, (mg + 1) * mg_mc)
            a_k = []
            for kk in range(k_dim):
                a_t = adj_pool.tile([P, mg_mc * ns], dt.bfloat16,
                                    tag=f"adj{kk}", name=f"adj_t{kk}")
                # adj stream rides the ACT HWDGE ring, separate from the
                # sync ring that carries x/consts/stores
                eng = nc.scalar if adj_bf16_host else nc.gpsimd
                eng.dma_start(
                    out=a_t[:].rearrange("p (c n) -> p c n", c=mg_mc),
                    in_=adjt[kk, :, cs, :],
                )
                a_k.append(a_t)
            # comb = conv_w[0]*a0 + conv_w[1]*a1  (bf16, DVE)
            nc.vector.tensor_scalar(
                a_k[1][:], a_k[1][:], cw_sb[:, 1:2], None, ALU.mult
            )
            comb = comb_pool.tile([P, mg_mc * ns], dt.bfloat16, tag="comb")
            nc.vector.scalar_tensor_tensor(
                comb[:], a_k[0][:], cw_sb[:, 0:1], a_k[1][:], ALU.mult, ALU.add
            )
            for c in range(mg_mc):
                mc = mg * mg_mc + c
                for h in range(nh):
                    lhsT = x4[:, mc, 2 * h : 2 * h + 2, :].rearrange(
                        "p b f -> p (b f)"
                    )
                    for s in range(nw):
                        nc.tensor.matmul(
                            psum_T[h][:, s * 512 : (s + 1) * 512],
                            lhsT,
                            comb[:, c * ns + s * 512 : c * ns + (s + 1) * 512],
                            start=(mc == 0),
                            stop=(mc == mc_cnt - 1),
                            skip_group_check=True,
                        )

        # ---------------- epilogue ----------------
        # out = tanh(0.5*siga*(y + cb) + xw + x0*sigb), all in [bf, n] layout
        for h in range(nh):
            e1 = work.tile([P, ns], dt.float32, tag="ew")
            nc.vector.tensor_tensor(e1[:], x0_sb[:, h, :], sigbB[:], ALU.mult)
            e2 = work.tile([P, ns], dt.float32, tag="ew")
            nc.vector.tensor_tensor(e2[:], e1[:], pxw_T[h][:], ALU.add)
            e3 = work.tile([P, ns], dt.float32, tag="ew")
            nc.vector.tensor_tensor(e3[:], psum_T[h][:], siga05B[:], ALU.mult)
            e4 = work.tile([P, ns], dt.float32, tag="ew")
            nc.vector.tensor_tensor(e4[:], e3[:], e2[:], ALU.add)
            # + 0.5*siga*conv_b
            e5 = work.tile([P, ns], dt.float32, tag="ew")
            nc.vector.scalar_tensor_tensor(
                e5[:], siga05B[:], cb_sb[:, 0:1], e4[:], ALU.mult, ALU.add
            )
            outt = outp.tile([P, ns], dt.float32, tag="outt")
            nc.scalar.activation(outt[:], e5[:], AF.Tanh)
            nc.sync.dma_start(out=y_T[h, :, :], in_=outt[:])

    nc.finalize()
    return nc


_NC_CACHE = {}


def _get_nc():
    key = (N, N_CORES, B, F, K, ADJ_BF16_HOST)
    if key not in _NC_CACHE:
        _NC_CACHE[key] = build_kernel(
            n=N, n_cores=N_CORES, b=B, f=F, k_dim=K, adj_bf16_host=ADJ_BF16_HOST
        )
    return _NC_CACHE[key]


def make_in_maps(x, x0, adj, alpha, beta, w, d, conv_w, conv_b, n_cores=N_CORES):
    """Host-side staging: slice rows per core and retile/transpose (pure
    layout transforms)."""
    k_dim, n, _ = adj.shape
    b, _, f = x.shape
    ns = n // n_cores
    mc_cnt = n // P
    nh = (b * f) // P
    f32 = np.float32
    adj = np.asarray(adj, dtype=f32)
    x = np.asarray(x, dtype=f32)
    x0 = np.asarray(x0, dtype=f32)
    alpha = np.asarray(alpha, dtype=f32)
    beta = np.asarray(beta, dtype=f32)

    # xres_in[p, mc, b, f] = x[b, mc*128+p, f]   (shared by all cores)
    xres_in = np.ascontiguousarray(
        x.reshape(b, mc_cnt, P, f).transpose(2, 1, 0, 3)
    )
    if ADJ_BF16_HOST:
        import ml_dtypes

        xres_in = xres_in.astype(ml_dtypes.bfloat16)
    wT = np.ascontiguousarray(np.asarray(w, dtype=f32).T)

    in_maps = []
    for c in range(n_cores):
        rows = slice(c * ns, (c + 1) * ns)
        ac = adj[:, rows, :]  # [k, ns, n] view
        # adjt[k, p, mc, nn] = ac[k, nn, mc*128+p]
        s0, s1, s2 = ac.strides
        adjt = np.lib.stride_tricks.as_strided(
            ac, shape=(k_dim, P, mc_cnt, ns), strides=(s0, s2, P * s2, s1)
        )
        adjt = np.ascontiguousarray(adjt)
        if ADJ_BF16_HOST:
            import ml_dtypes

            adjt = adjt.astype(ml_dtypes.bfloat16)

        xr = x[:, rows, :]  # [b, ns, f]
        xt_rows = np.ascontiguousarray(xr.transpose(2, 0, 1))  # [f, b, ns]
        # x0t_in[p, h, nn] = x0[2h + p//64, rows0+nn, p%64]
        x0t_in = np.ascontiguousarray(
            x0[:, rows, :].transpose(0, 2, 1).reshape(nh, P, ns).transpose(1, 0, 2)
        )
        ab = np.stack([alpha[rows], beta[rows]], axis=0)

        in_maps.append(
            {
                "adjt": adjt,
                "xres_in": xres_in,
                "xt_rows": xt_rows,
                "x0t_in": x0t_in,
                "ab_rows": np.ascontiguousarray(ab),
                "wT": wT,
                "d": np.ascontiguousarray(d, dtype=f32),
                "conv_w": np.ascontiguousarray(conv_w, dtype=f32),
                "conv_b": np.ascontiguousarray(conv_b, dtype=f32),
            }
        )
    return in_maps


def assemble_output(per_core_y, n_cores=N_CORES):
    """y_T[h, p, n] per core -> full [b, n, f]."""
    parts = []
    for c in range(n_cores):
        yt = per_core_y[c]  # [nh, P, ns]
        nh_, p_, ns_ = yt.shape
        b_ = nh_ * (p_ // 64)
        f_ = 64
        # [nh, P, ns] -> [b, f, ns] -> [b, ns, f]
        parts.append(
            yt.reshape(nh_, p_ // f_, f_, ns_)
            .reshape(b_, f_, ns_)
            .transpose(0, 2, 1)
        )
    return np.concatenate(parts, axis=1)


def kernel(x, x0, adj, alpha, beta, w, d, conv_w, conv_b):
    x = np.asarray(x)
    x0 = np.asarray(x0)
    adj = np.asarray(adj)
    alpha = np.asarray(alpha)
    beta = np.asarray(beta)
    w = np.asarray(w)
    d = np.asarray(d)
    conv_w = np.asarray(conv_w)
    conv_b = np.asarray(conv_b)

    nc = _get_nc()
    in_maps = make_in_maps(x, x0, adj, alpha, beta, w, d, conv_w, conv_b)
    res = run_bass_kernel_spmd(nc, in_maps, core_ids=list(range(N_CORES)))
    out = assemble_output([res.results[c]["y_T"] for c in range(N_CORES)])
    return out.astype(np.float32)
